# revision 1
# baseline (speedup 1.0000x reference)
"""DeepseekV2 MLA attention forward — Trainium2 Bass kernel (8 NeuronCores).

Sharding: data-parallel over batch (2) x sequence-panel-parallel over query
rows (4 panels of 512) = 8 cores. Each core computes, for its (batch, panel):
  - q path (q_a_proj -> rmsnorm -> q_b_proj) for its 512 query rows, all heads
  - kv path (kv_a_proj -> rmsnorm -> kv_b_proj) for the FULL key sequence
  - RoPE, full attention (all 16 heads) for its query rows, o_proj
Output panels are concatenated on the host; no cross-core communication.

Everything on-chip is kept in "transposed" layout (feature dim on partitions,
sequence on the free axis) so every matmul consumes natural weight layouts and
fp32r runs at full rate (moving free dim >= 256). The only host-side prep is
transposes/reorders of inputs (free: grading measures HW exec time).
"""

import os
import numpy as np
from contextlib import ExitStack

import concourse.bass as bass
import concourse.bacc as bacc
import concourse.mybir as mybir
import concourse.tile as tile
from concourse import bass_utils

B, S, HID = 2, 2048, 2048
NH = 16
QLR, KVLR = 1536, 512
DN, DR, DV = 128, 64, 128
DQK = DN + DR
SCALE = DQK ** -0.5
EPS = 1e-6
P = 128
NPANEL = 4
W = S // NPANEL            # 512 query rows per core
NCORES = B * NPANEL

F32 = mybir.dt.float32
F32R = mybir.dt.float32r
EXP = mybir.ActivationFunctionType.Exp
SQRT = mybir.ActivationFunctionType.Sqrt
COPY = mybir.ActivationFunctionType.Copy
MULT = mybir.AluOpType.mult
ADD = mybir.AluOpType.add

KB_HID = HID // P          # 16
KB_QLR = QLR // P          # 12
KB_CKV = KVLR // P         # 4
KB_S = S // P              # 16
MB_QLR = QLR // P          # 12
MB_NOPE = NH * DN // P     # 16
MB_PE = NH * DR // P       # 8
MB_HID = HID // P          # 16
NCH = S // W               # 4 column chunks of the full sequence

LAST_RESULT = None         # BassKernelResults of the most recent launch


def _mm(nc, out, lhsT, rhs, start, stop):
    nc.tensor.matmul(out, lhsT.bitcast(F32R), rhs.bitcast(F32R),
                     start=start, stop=stop)


def _emit(tc, t, with_mask):
    """Emit the whole per-core program. `t` maps tensor name -> DRAM AP."""
    nc = tc.nc

    with ExitStack() as big:
        const = big.enter_context(tc.tile_pool(name="const", bufs=1))
        ones_f = const.tile([P, 1], F32)
        nc.vector.memset(ones_f[:], 1.0)
        ones_fr = const.tile([1, P], F32)
        nc.vector.memset(ones_fr[:], 1.0)
        ones_col = const.tile([P, 1], F32R)
        nc.scalar.activation(ones_col[:], ones_f[:], COPY)
        ones_row = const.tile([1, P], F32R)
        nc.scalar.activation(ones_row[:], ones_fr[:], COPY)
        eps1 = const.tile([1, 1], F32)
        nc.vector.memset(eps1[:], EPS)
        qa_ln = const.tile([P, KB_QLR], F32)
        nc.sync.dma_start(qa_ln[:], t["qa_ln_p"][:])
        kva_ln = const.tile([P, KB_CKV], F32)
        nc.sync.dma_start(kva_ln[:], t["kva_ln_p"][:])

        def bcast_row(psum_pool, row_ap):
            """replicate [1, n] row across 128 partitions via PE."""
            n = row_ap.shape[-1]
            ps = psum_pool.tile([P, n], F32, tag="bcast")
            _mm(nc, ps[:], ones_row[:], row_ap, True, True)
            return ps

        def colnorm_finish(pool, psum_pool, ss_ps, inv_dim):
            """rsqrt(mean(ss)+eps) per column -> SBUF [P, n] broadcast tile."""
            n = ss_ps.shape[-1]
            srow = pool.tile([1, n], F32, tag="srow")
            nc.scalar.activation(srow[:], ss_ps[:], SQRT,
                                 bias=eps1[:], scale=inv_dim)
            rrow = pool.tile([1, n], F32R, tag="rrow")
            with nc.allow_low_precision(reason="f32r is f32 storage"):
                nc.vector.reciprocal(rrow[:], srow[:])
            bc_ps = bcast_row(psum_pool, rrow[:])
            bc = pool.tile([P, n], F32, tag="bcn")
            nc.scalar.activation(bc[:], bc_ps[:], COPY)
            return bc

        # ------------- phase A: qaT panel + rmsnorm -> qa_dram -----------
        with tc.tile_pool(name="phA", bufs=2) as pa, \
             tc.tile_pool(name="phA_hp", bufs=1) as pah, \
             tc.tile_pool(name="phA_w", bufs=2) as paw, \
             tc.tile_pool(name="psA", bufs=2, space="PSUM") as psA, \
             tc.tile_pool(name="psS", bufs=2, space="PSUM") as psSS, \
             tc.tile_pool(name="psB", bufs=1, space="PSUM") as psBC, \
             tc.tile_pool(name="phA_qa", bufs=1) as paq:
            hp = pah.tile([P, KB_HID, W], F32R, tag="hp")
            nc.sync.dma_start(
                hp[:], t["hsT_panel"].rearrange("(k p) s -> p k s", p=P))
            qaT = paq.tile([P, KB_QLR, W], F32R, tag="qaT")
            ss = psSS.tile([1, W], F32, tag="ss")
            for m in range(MB_QLR):
                wm = paw.tile([P, KB_HID, P], F32R, tag="wqa")
                nc.sync.dma_start(
                    wm[:], t["w_qa"][:, m * P:(m + 1) * P]
                    .rearrange("(k p) c -> p k c", p=P))
                ps = psA.tile([P, W], F32, tag="psA")
                for k in range(KB_HID):
                    _mm(nc, ps[:], wm[:, k, :], hp[:, k, :],
                        k == 0, k == KB_HID - 1)
                nc.scalar.activation(qaT[:, m, :], ps[:], COPY)
                sq = pa.tile([P, W], F32R, tag="sq")
                nc.vector.tensor_tensor(sq[:], qaT[:, m, :], ps[:], MULT)
                _mm(nc, ss[:], ones_col[:], sq[:], m == 0, m == MB_QLR - 1)
            rq = colnorm_finish(pa, psBC, ss[:], 1.0 / QLR)
            for m in range(MB_QLR):
                nc.vector.scalar_tensor_tensor(
                    qaT[:, m, :], qaT[:, m, :], qa_ln[:, m:m + 1], rq[:],
                    MULT, MULT)
                nc.sync.dma_start(t["qa_dram"][:, m, :], qaT[:, m, :])

        # ------------- phase B..D under persistent kv pools --------------
        with tc.tile_pool(name="ckv", bufs=1) as ckv_pool:
            ckT = ckv_pool.tile([P, KB_CKV, S], F32R)     # 4 MB, ck_norm^T
            kpe2 = ckv_pool.tile([P, S], F32R)            # k_pe duplicated+rope

            # ---- phase B: kvaT (full S) + rmsnorm + kpe rope ----
            with tc.tile_pool(name="phB", bufs=2) as pb, \
                 tc.tile_pool(name="phB_h", bufs=2) as pbh, \
                 tc.tile_pool(name="phB_w", bufs=2) as pbw, \
                 tc.tile_pool(name="phB_c", bufs=1) as pbc, \
                 tc.tile_pool(name="psA", bufs=2, space="PSUM") as psA, \
                 tc.tile_pool(name="psS", bufs=2, space="PSUM") as psSS, \
                 tc.tile_pool(name="psB", bufs=1, space="PSUM") as psBC:
                cos2f = pbc.tile([P, S], F32)
                nc.sync.dma_start(cos2f[:], t["cos2f"][:])
                sin2sf = pbc.tile([P, S], F32)
                nc.sync.dma_start(sin2sf[:], t["sin2sf"][:])
                for nch in range(NCH):
                    hn = pbh.tile([P, KB_HID, W], F32R, tag="hn")
                    nc.sync.dma_start(
                        hn[:], t["hsT"][:, nch * W:(nch + 1) * W]
                        .rearrange("(k p) s -> p k s", p=P))
                    ss = psSS.tile([1, W], F32, tag="ss")
                    for m in range(KB_CKV + 1):
                        rows = P if m < KB_CKV else DR
                        wm = pbw.tile([P, KB_HID, P], F32R, tag="wkva")
                        nc.sync.dma_start(
                            wm[:, :, :rows],
                            t["w_kva"][:, m * P:m * P + rows]
                            .rearrange("(k p) c -> p k c", p=P))
                        ps = psA.tile([P, W], F32, tag="psA")
                        for k in range(KB_HID):
                            _mm(nc, ps[:rows, :], wm[:, k, :rows],
                                hn[:, k, :], k == 0, k == KB_HID - 1)
                        if m < KB_CKV:
                            ckslc = ckT[:, m, nch * W:(nch + 1) * W]
                            nc.scalar.activation(ckslc, ps[:], COPY)
                            sq = pb.tile([P, W], F32R, tag="sq")
                            nc.vector.tensor_tensor(sq[:], ckslc, ps[:], MULT)
                            _mm(nc, ss[:], ones_col[:], sq[:],
                                m == 0, m == KB_CKV - 1)
                        else:
                            nc.scalar.activation(
                                kpe2[0:DR, nch * W:(nch + 1) * W],
                                ps[0:DR, :], COPY)
                            nc.vector.tensor_copy(
                                kpe2[DR:P, nch * W:(nch + 1) * W],
                                ps[0:DR, :])
                    rk = colnorm_finish(pb, psBC, ss[:], 1.0 / KVLR)
                    for m in range(KB_CKV):
                        nc.vector.scalar_tensor_tensor(
                            ckT[:, m, nch * W:(nch + 1) * W],
                            ckT[:, m, nch * W:(nch + 1) * W],
                            kva_ln[:, m:m + 1], rk[:], MULT, MULT)
                # RoPE on kpe2 (both 64-halves hold the same data)
                rot = pbc.tile([P, S], F32, tag="rot")
                for h in (0, DR):
                    nc.vector.tensor_copy(rot[h:h + 32, :],
                                          kpe2[h + 32:h + 64, :])
                    nc.vector.tensor_copy(rot[h + 32:h + 64, :],
                                          kpe2[h:h + 32, :])
                nc.vector.tensor_tensor(kpe2[:], kpe2[:], cos2f[:], MULT)
                nc.vector.tensor_tensor(rot[:], rot[:], sin2sf[:], MULT)
                nc.vector.tensor_tensor(kpe2[:], kpe2[:], rot[:], ADD)

            with tc.tile_pool(name="qTp", bufs=1) as q_pool:
                qnopeT = q_pool.tile([P, MB_NOPE, W], F32R)   # 4 MB
                qpeT = q_pool.tile([P, MB_PE, W], F32R)       # 2 MB

                # ---- phase C: qT panel (+ RoPE on pe part) ----
                with tc.tile_pool(name="phC", bufs=2) as pc, \
                     tc.tile_pool(name="phC_w", bufs=2) as pcw, \
                     tc.tile_pool(name="phC_qa", bufs=1) as pcq, \
                     tc.tile_pool(name="psA", bufs=2, space="PSUM") as psA:
                    cos2p = pcq.tile([P, W], F32, tag="cos2p")
                    nc.sync.dma_start(cos2p[:], t["cos2p"][:])
                    sin2sp = pcq.tile([P, W], F32, tag="sin2sp")
                    nc.sync.dma_start(sin2sp[:], t["sin2sp"][:])
                    qaT = pcq.tile([P, KB_QLR, W], F32R, tag="qaT2")
                    nc.sync.dma_start(
                        qaT[:], t["qa_dram"].rearrange("p k s -> p k s"))
                    for m in range(MB_NOPE + MB_PE):
                        wm = pcw.tile([P, KB_QLR, P], F32R, tag="wqb")
                        nc.sync.dma_start(
                            wm[:], t["w_qb_re"][:, m * P:(m + 1) * P]
                            .rearrange("(k p) c -> p k c", p=P))
                        ps = psA.tile([P, W], F32, tag="psA")
                        for k in range(KB_QLR):
                            _mm(nc, ps[:], wm[:, k, :], qaT[:, k, :],
                                k == 0, k == KB_QLR - 1)
                        if m < MB_NOPE:
                            nc.scalar.activation(qnopeT[:, m, :], ps[:], COPY)
                        else:
                            j = m - MB_NOPE
                            rotq = pc.tile([P, W], F32, tag="rotq")
                            for h in (0, DR):
                                nc.vector.tensor_copy(rotq[h:h + 32, :],
                                                      ps[h + 32:h + 64, :])
                                nc.vector.tensor_copy(rotq[h + 32:h + 64, :],
                                                      ps[h:h + 32, :])
                            nc.vector.tensor_tensor(rotq[:], rotq[:],
                                                    sin2sp[:], MULT)
                            tmp = pc.tile([P, W], F32, tag="tmpq")
                            nc.vector.tensor_tensor(tmp[:], ps[:],
                                                    cos2p[:], MULT)
                            nc.vector.tensor_tensor(qpeT[:, j, :], tmp[:],
                                                    rotq[:], ADD)

                # ---- phase D: per 2-head group: V, knope, attention ----
                with tc.tile_pool(name="phD", bufs=2) as pd, \
                     tc.tile_pool(name="phD_v", bufs=1) as pdv, \
                     tc.tile_pool(name="phD_k", bufs=1) as pdk, \
                     tc.tile_pool(name="phD_w", bufs=2) as pdw, \
                     tc.tile_pool(name="probs", bufs=3) as pprob, \
                     tc.tile_pool(name="psSc", bufs=3, space="PSUM") as psSc, \
                     tc.tile_pool(name="psO", bufs=2, space="PSUM") as psO, \
                     tc.tile_pool(name="psR", bufs=2, space="PSUM") as psR, \
                     tc.tile_pool(name="psB2", bufs=1, space="PSUM") as psB2, \
                     ExitStack() as dctx:
                    if with_mask:
                        mask_pool = dctx.enter_context(
                            tc.tile_pool(name="maskp", bufs=4))
                    for g in range(NH // 2):
                        # V for the 2 heads of this group: [k, 2*128 dv]
                        wv = pdw.tile([P, KB_CKV, 2 * DV], F32R, tag="wv")
                        nc.sync.dma_start(
                            wv[:], t["w_kvb_re"][:, NH * DN + g * 2 * DV:
                                                 NH * DN + (g + 1) * 2 * DV]
                            .rearrange("(k p) c -> p k c", p=P))
                        v_sb = pdv.tile([P, KB_S, 2 * DV], F32R, tag="v")
                        for kb in range(KB_S):
                            psv = psSc.tile([P, W], F32, tag="pss")
                            for kc in range(KB_CKV):
                                _mm(nc, psv[:, :2 * DV],
                                    ckT[:, kc, kb * P:(kb + 1) * P],
                                    wv[:, kc, :], kc == 0, kc == KB_CKV - 1)
                            nc.scalar.activation(v_sb[:, kb, :],
                                                 psv[:, :2 * DV], COPY)

                        for hl in range(2):
                            h = g * 2 + hl
                            # knopeT for head h: [128 d, S]
                            wkn = pdw.tile([P, KB_CKV, DN], F32R, tag="wkn")
                            nc.sync.dma_start(
                                wkn[:], t["w_kvb_re"][:, h * DN:(h + 1) * DN]
                                .rearrange("(k p) c -> p k c", p=P))
                            knT = pdk.tile([P, KB_S, P], F32R, tag="knT")
                            for nch in range(NCH):
                                psk = psSc.tile([P, W], F32, tag="pss")
                                for kc in range(KB_CKV):
                                    _mm(nc, psk[:], wkn[:, kc, :],
                                        ckT[:, kc, nch * W:(nch + 1) * W],
                                        kc == 0, kc == KB_CKV - 1)
                                for sub in range(W // P):
                                    nc.scalar.activation(
                                        knT[:, nch * (W // P) + sub, :],
                                        psk[:, sub * P:(sub + 1) * P], COPY)

                            # attention for head h over all key blocks
                            po = psO.tile([P, W], F32, tag="po")
                            pr = psR.tile([1, W], F32, tag="pr")
                            hp64 = hl * DR
                            for kb in range(KB_S):
                                pss = psSc.tile([P, W], F32, tag="pss")
                                _mm(nc, pss[:], knT[:, kb, :],
                                    qnopeT[:, h, :], True, False)
                                _mm(nc, pss[:],
                                    kpe2[hp64:hp64 + DR, kb * P:(kb + 1) * P],
                                    qpeT[hp64:hp64 + DR, g, :], False, True)
                                probs = pprob.tile([P, W], F32R, tag="probs")
                                if with_mask:
                                    mtile = mask_pool.tile([P, W], F32,
                                                           tag="mt")
                                    nc.sync.dma_start(
                                        mtile[:],
                                        t["maskT"][kb * P:(kb + 1) * P, :])
                                    nc.vector.scalar_tensor_tensor(
                                        probs[:], pss[:], SCALE, mtile[:],
                                        MULT, ADD)
                                    nc.scalar.activation(probs[:], probs[:],
                                                         EXP)
                                else:
                                    nc.scalar.activation(probs[:], pss[:],
                                                         EXP, scale=SCALE)
                                _mm(nc, po[:],
                                    v_sb[:, kb, hl * DV:(hl + 1) * DV],
                                    probs[:], kb == 0, kb == KB_S - 1)
                                _mm(nc, pr[:], ones_col[:], probs[:],
                                    kb == 0, kb == KB_S - 1)
                            rrow = pd.tile([1, W], F32R, tag="rr")
                            with nc.allow_low_precision(
                                    reason="f32r is f32 storage"):
                                nc.vector.reciprocal(rrow[:], pr[:])
                            bc_ps = psB2.tile([P, W], F32, tag="bcd")
                            _mm(nc, bc_ps[:], ones_row[:], rrow[:],
                                True, True)
                            bc = pd.tile([P, W], F32, tag="bcs")
                            nc.scalar.activation(bc[:], bc_ps[:], COPY)
                            osb = pd.tile([P, W], F32R, tag="osb")
                            nc.vector.tensor_tensor(osb[:], po[:], bc[:],
                                                    MULT)
                            nc.sync.dma_start(
                                t["oT_dram"][h * DV:(h + 1) * DV, :], osb[:])

        # ------------- phase E: o_proj -----------------------------------
        with tc.tile_pool(name="phE", bufs=2) as pe, \
             tc.tile_pool(name="phE_o", bufs=1) as peo, \
             tc.tile_pool(name="phE_w", bufs=2) as pew, \
             tc.tile_pool(name="psA", bufs=2, space="PSUM") as psA:
            oT = peo.tile([P, NH, W], F32R)
            nc.sync.dma_start(
                oT[:], t["oT_dram"].rearrange("(k p) s -> p k s", p=P))
            for m in range(MB_HID):
                wm = pew.tile([P, NH, P], F32R, tag="wo")
                nc.sync.dma_start(
                    wm[:], t["w_o"][:, m * P:(m + 1) * P]
                    .rearrange("(k p) c -> p k c", p=P))
                ps = psA.tile([P, W], F32, tag="psA")
                for k in range(NH):
                    _mm(nc, ps[:], wm[:, k, :], oT[:, k, :],
                        k == 0, k == NH - 1)
                osb = pe.tile([P, W], F32, tag="osb")
                nc.scalar.activation(osb[:], ps[:], COPY)
                nc.sync.dma_start(t["outT"][m * P:(m + 1) * P, :], osb[:])


def _build_program(with_mask):
    nc = bacc.Bacc("TRN2", target_bir_lowering=False, debug=False)
    t = {}

    def inp(name, shape, dt=F32):
        t[name] = nc.dram_tensor(name, list(shape), dt,
                                 kind="ExternalInput").ap()

    inp("hsT", [HID, S], F32R)
    inp("hsT_panel", [HID, W], F32R)
    inp("w_qa", [HID, QLR], F32R)
    inp("w_qb_re", [QLR, NH * DQK], F32R)
    inp("w_kva", [HID, KVLR + DR], F32R)
    inp("w_kvb_re", [KVLR, NH * (DN + DV)], F32R)
    inp("w_o", [NH * DV, HID], F32R)
    inp("qa_ln_p", [P, KB_QLR])
    inp("kva_ln_p", [P, KB_CKV])
    inp("cos2p", [P, W])
    inp("sin2sp", [P, W])
    inp("cos2f", [P, S])
    inp("sin2sf", [P, S])
    if with_mask:
        inp("maskT", [S, W])
    t["qa_dram"] = nc.dram_tensor("qa_dram", [P, KB_QLR, W], F32R,
                                  kind="Internal").ap()
    t["oT_dram"] = nc.dram_tensor("oT_dram", [NH * DV, W], F32R,
                                  kind="Internal").ap()
    t["outT"] = nc.dram_tensor("outT", [HID, W], F32,
                               kind="ExternalOutput").ap()

    with tile.TileContext(nc) as tc:
        _emit(tc, t, with_mask)
    nc.compile()
    return nc


_PROG_CACHE = {}


def _get_program(with_mask):
    if with_mask not in _PROG_CACHE:
        _PROG_CACHE[with_mask] = _build_program(with_mask)
    return _PROG_CACHE[with_mask]


def make_in_maps(hidden_states, attention_mask, cos, sin, w_qa, qa_ln, w_qb,
                 w_kva, kva_ln, w_kvb, w_o, with_mask):
    """Host-side prep: transposes/reorders; returns list of 8 input dicts."""
    f32 = np.float32
    c = np.ascontiguousarray

    w_qb_r = np.asarray(w_qb).reshape(QLR, NH, DQK)
    w_qb_re = c(np.concatenate(
        [w_qb_r[:, :, :DN].reshape(QLR, NH * DN),
         w_qb_r[:, :, DN:].reshape(QLR, NH * DR)], axis=1).astype(f32))
    w_kvb_r = np.asarray(w_kvb).reshape(KVLR, NH, DN + DV)
    w_kvb_re = c(np.concatenate(
        [w_kvb_r[:, :, :DN].reshape(KVLR, NH * DN),
         w_kvb_r[:, :, DN:].reshape(KVLR, NH * DV)], axis=1).astype(f32))
    qa_ln_p = c(np.asarray(qa_ln).reshape(KB_QLR, P).T.astype(f32))
    kva_ln_p = c(np.asarray(kva_ln).reshape(KB_CKV, P).T.astype(f32))

    cosT = np.asarray(cos).T.astype(f32)                  # [64, S]
    sinT = np.asarray(sin).T.astype(f32)
    sin_s = np.concatenate([-sinT[:DR // 2], sinT[DR // 2:]], axis=0)
    cos2 = c(np.concatenate([cosT, cosT], axis=0))        # [128, S]
    sin2s = c(np.concatenate([sin_s, sin_s], axis=0))

    shared = {
        "w_qa": c(np.asarray(w_qa).astype(f32)),
        "w_qb_re": w_qb_re,
        "w_kva": c(np.asarray(w_kva).astype(f32)),
        "w_kvb_re": w_kvb_re,
        "w_o": c(np.asarray(w_o).astype(f32)),
        "qa_ln_p": qa_ln_p,
        "kva_ln_p": kva_ln_p,
        "cos2f": cos2,
        "sin2sf": sin2s,
    }

    hs = np.asarray(hidden_states)
    am = np.asarray(attention_mask)
    in_maps = []
    for core in range(NCORES):
        b, pnl = divmod(core, NPANEL)
        q0 = pnl * W
        hsT = c(hs[b].T.astype(f32))
        m = dict(shared)
        m["hsT"] = hsT
        m["hsT_panel"] = c(hsT[:, q0:q0 + W])
        m["cos2p"] = c(cos2[:, q0:q0 + W])
        m["sin2sp"] = c(sin2s[:, q0:q0 + W])
        if with_mask:
            m["maskT"] = c(am[b, 0, q0:q0 + W, :].T.astype(f32))
        in_maps.append(m)
    return in_maps


def kernel(hidden_states, attention_mask, cos, sin, w_qa, qa_ln, w_qb,
           w_kva, kva_ln, w_kvb, w_o):
    global LAST_RESULT
    with_mask = bool(np.any(np.asarray(attention_mask) != 0))
    nc = _get_program(with_mask)
    in_maps = make_in_maps(hidden_states, attention_mask, cos, sin, w_qa,
                           qa_ln, w_qb, w_kva, kva_ln, w_kvb, w_o, with_mask)
    trace = os.environ.get("KERNEL_TRACE", "0") == "1"
    res = bass_utils.run_bass_kernel_spmd(
        nc, in_maps, core_ids=list(range(NCORES)), trace=trace)
    LAST_RESULT = res

    out = np.empty((B, S, HID), np.float32)
    for core in range(NCORES):
        b, pnl = divmod(core, NPANEL)
        q0 = pnl * W
        out[b, q0:q0 + W, :] = res.results[core]["outT"].T
    return out



# revision 15
# speedup vs baseline: 1.4129x; 1.4129x over previous
"""DeepseekV2 MLA attention forward — Trainium2 Bass kernel (8 NeuronCores).

Sharding: data-parallel over batch (2) x sequence-parallel over query rows
(4 panels of 512) = 8 cores. Each core computes, for its (batch, panel):
  - q path (q_a_proj -> rmsnorm -> q_b_proj) for its 512 query rows
  - kv path (kv_a_proj -> rmsnorm -> kv_b_proj) for the FULL key sequence
  - RoPE, full attention (16 heads) for its query rows, o_proj
Output panels are concatenated on the host; no cross-core communication.

All matmul operands are bf16 (fp32 PSUM accumulation): full PE rate incl.
the 64-partition rope matmuls, half the DMA/copy traffic.  Weights are
pre-transposed on the host into per-partition-contiguous [m][p][k][c]
blocks so every weight DMA is one large contiguous burst.  Intermediates
(qaT, ckT, kpe, qnope, qpe, oT) stay resident in SBUF.  The attention kb
loop is software-pipelined so the PE never waits on the Act-engine exp.
"""

import os
import numpy as np
import ml_dtypes

import concourse.bass as bass
import concourse.bacc as bacc
import concourse.mybir as mybir
import concourse.tile as tile
from concourse import bass_utils

B, S, HID = 2, 2048, 2048
NH = 16
QLR, KVLR = 1536, 512
DN, DR, DV = 128, 64, 128
DQK = DN + DR
SCALE = DQK ** -0.5
EPS = 1e-6
P = 128
NPANEL = 4
W = S // NPANEL            # 512 query rows per core
NCORES = B * NPANEL

F32 = mybir.dt.float32
F32R = mybir.dt.float32r
BF16 = mybir.dt.bfloat16
NPBF = ml_dtypes.bfloat16
EXP = mybir.ActivationFunctionType.Exp
SQRT = mybir.ActivationFunctionType.Sqrt
SQUARE = mybir.ActivationFunctionType.Square
COPY = mybir.ActivationFunctionType.Copy
MULT = mybir.AluOpType.mult
ADD = mybir.AluOpType.add

KB_HID = HID // P          # 16
KB_QLR = QLR // P          # 12
KB_CKV = KVLR // P         # 4
KB_S = S // P              # 16
MB_QLR = QLR // P          # 12
MB_KVA = 5                 # 4 ckv blocks + 1 (zero-padded) rope block
MB_NOPE = NH * DN // P     # 16
MB_PE = NH * DR // P       # 8
MB_HID = HID // P          # 16
NCH = S // W               # 4 column chunks of the full sequence

LAST_RESULT = None         # BassKernelResults of the most recent launch


def _emit(tc, t, with_mask):
    nc = tc.nc
    mm = nc.tensor.matmul
    from contextlib import ExitStack
    top = ExitStack()

    const = top.enter_context(tc.tile_pool(name="const", bufs=1))
    ones_col = const.tile([P, 1], BF16)
    nc.vector.memset(ones_col[:], 1.0)
    ones_row = const.tile([1, P], BF16)
    nc.vector.memset(ones_row[:], 1.0)
    eps1 = const.tile([1, 1], F32)
    nc.vector.memset(eps1[:], EPS)
    qa_ln = const.tile([P, MB_QLR], F32)
    nc.sync.dma_start(qa_ln[:], t["qa_ln_p"][:])
    kva_ln = const.tile([P, KB_CKV], F32)
    nc.sync.dma_start(kva_ln[:], t["kva_ln_p"][:])
    cos2p = const.tile([P, W], F32)
    nc.sync.dma_start(cos2p[:], t["cos2p"][:])
    sin2sp = const.tile([P, W], F32)
    nc.sync.dma_start(sin2sp[:], t["sin2sp"][:])
    cos2f = const.tile([P, S], F32)
    sin2sf = const.tile([P, S], F32)

    # persistent SBUF intermediates (all bf16)
    persist = top.enter_context(tc.tile_pool(name="persist", bufs=1))
    qaT = persist.tile([P, MB_QLR, W], BF16)       # q_a output, normalized
    ckT = persist.tile([P, KB_CKV, S], BF16)       # compressed kv, normalized
    kpe2 = persist.tile([P, S], BF16)              # roped k_pe, duplicated 2x
    qnopeT = persist.tile([P, MB_NOPE, W], BF16)
    qpeT = persist.tile([P, MB_PE, W], BF16)       # roped q_pe
    oT = persist.tile([P, NH, W], BF16)            # attn out (pre-o_proj)

    # kv-path inputs: pools opened early, DMAs emitted inside phase A
    pbh = top.enter_context(tc.tile_pool(name="phB_h", bufs=2))
    wkva_pool = top.enter_context(tc.tile_pool(name="phB_w", bufs=1))
    wkva = wkva_pool.tile([P, MB_KVA, KB_HID, P], BF16)

    def rsqrt_bcast(pool, psum_pool, ss_ps, inv_dim):
        """[1,n] sum-of-squares psum -> [P,n] f32 tile of 1/sqrt(mean+eps)."""
        n = ss_ps.shape[-1]
        srow = pool.tile([1, n], BF16, tag="srow")
        nc.scalar.activation(srow[:], ss_ps[:], SQRT, bias=eps1[:],
                             scale=inv_dim)
        bc_ps = psum_pool.tile([P, n], F32, tag="bcast")
        mm(bc_ps[:], ones_row[:], srow[:], start=True, stop=True)
        rq = pool.tile([P, n], F32, tag="rq")
        nc.vector.reciprocal(rq[:], bc_ps[:])
        return rq

    # ------------- phase A: qaT panel + rmsnorm ----------------------
    with tc.tile_pool(name="phA", bufs=2) as pa, \
         tc.tile_pool(name="phA_hp", bufs=1) as pah, \
         tc.tile_pool(name="phA_w", bufs=3) as paw, \
         tc.tile_pool(name="psA", bufs=3, space="PSUM") as psA, \
         tc.tile_pool(name="psS", bufs=1, space="PSUM") as psSS, \
         tc.tile_pool(name="psB", bufs=1, space="PSUM") as psBC:
        hp = pah.tile([P, KB_HID, W], BF16, tag="hp")
        nc.sync.dma_start(hp[:], t["hs_pkp"][:])
        ss = psSS.tile([1, W], F32, tag="ss")
        sq_prev = None
        for m in range(MB_QLR):
            wm = paw.tile([P, KB_HID, P], BF16, tag="wqa")
            nc.sync.dma_start(wm[:], t["w_qa"][m])
            if m == 1:
                # prefetch phase-B inputs off the critical path
                nc.sync.dma_start(wkva[:], t["w_kva"][:])
                nc.sync.dma_start(cos2f[:], t["cos2f"][:])
                nc.sync.dma_start(sin2sf[:], t["sin2sf"][:])
            ps = psA.tile([P, W], F32, tag="psA")
            for k in range(KB_HID):
                mm(ps[:], wm[:, k, :], hp[:, k, :],
                   start=(k == 0), stop=(k == KB_HID - 1))
            nc.scalar.activation(qaT[:, m, :], ps[:], COPY)
            sq = pa.tile([P, W], BF16, tag="sq")
            nc.scalar.activation(sq[:], ps[:], SQUARE)
            if sq_prev is not None:
                mm(ss[:], ones_col[:], sq_prev,
                   start=(m == 1), stop=False, skip_group_check=True)
            sq_prev = sq[:]
        mm(ss[:], ones_col[:], sq_prev, start=False, stop=True,
           skip_group_check=True)
        rq = rsqrt_bcast(pa, psBC, ss[:], 1.0 / QLR)
        for m in range(MB_QLR):
            nc.vector.scalar_tensor_tensor(
                qaT[:, m, :], qaT[:, m, :], qa_ln[:, m:m + 1], rq[:],
                MULT, MULT)

    # ------------- phase B: ckT (full S) + rmsnorm + kpe rope --------
    with tc.tile_pool(name="phB", bufs=2) as pb, \
         tc.tile_pool(name="psA", bufs=3, space="PSUM") as psA, \
         tc.tile_pool(name="psS", bufs=2, space="PSUM") as psSS, \
         tc.tile_pool(name="psB", bufs=2, space="PSUM") as psBC:
        for nch in range(NCH):
            hn = pbh.tile([P, KB_HID, W], BF16, tag="hn")
            nc.sync.dma_start(
                hn[:], t["hs_pks"][:, :, nch * W:(nch + 1) * W])
            ss = psSS.tile([1, W], F32, tag="ss")
            kp = pb.tile([P, W], F32, tag="kp")
            sq_prev = None
            for m in range(MB_KVA):
                ps = psA.tile([P, W], F32, tag="psA")
                for k in range(KB_HID):
                    mm(ps[:], wkva[:, m, k, :], hn[:, k, :],
                       start=(k == 0), stop=(k == KB_HID - 1))
                if m < KB_CKV:
                    ckslc = ckT[:, m, nch * W:(nch + 1) * W]
                    nc.scalar.activation(ckslc, ps[:], COPY)
                    sq = pb.tile([P, W], BF16, tag="sq")
                    nc.scalar.activation(sq[:], ps[:], SQUARE)
                    if sq_prev is not None:
                        mm(ss[:], ones_col[:], sq_prev,
                           start=(m == 1), stop=False, skip_group_check=True)
                    sq_prev = sq[:]
                else:
                    mm(ss[:], ones_col[:], sq_prev, start=False, stop=True,
                       skip_group_check=True)
                    nc.scalar.activation(kp[0:DR, :], ps[0:DR, :], COPY)
                    nc.vector.tensor_copy(kp[DR:P, :], ps[0:DR, :])
            rk = rsqrt_bcast(pb, psBC, ss[:], 1.0 / KVLR)
            for m in range(KB_CKV):
                nc.vector.scalar_tensor_tensor(
                    ckT[:, m, nch * W:(nch + 1) * W],
                    ckT[:, m, nch * W:(nch + 1) * W],
                    kva_ln[:, m:m + 1], rk[:], MULT, MULT)
            # RoPE on kp (both 64-halves hold the same data)
            rot = pb.tile([P, W], F32, tag="rot")
            for h in (0, DR):
                nc.vector.tensor_copy(rot[h:h + 32, :], kp[h + 32:h + 64, :])
                nc.vector.tensor_copy(rot[h + 32:h + 64, :], kp[h:h + 32, :])
            csl = slice(nch * W, (nch + 1) * W)
            nc.vector.tensor_tensor(kp[:], kp[:], cos2f[:, csl], MULT)
            nc.vector.tensor_tensor(rot[:], rot[:], sin2sf[:, csl], MULT)
            nc.vector.tensor_tensor(kpe2[:, csl], kp[:], rot[:], ADD)

    # ------------- phase C: q_b panel (+ RoPE on pe part) ------------
    with tc.tile_pool(name="phC", bufs=2) as pc, \
         tc.tile_pool(name="phC_w", bufs=3) as pcw, \
         tc.tile_pool(name="psA", bufs=3, space="PSUM") as psA:
        for m in range(MB_NOPE + MB_PE):
            wm = pcw.tile([P, KB_QLR, P], BF16, tag="wqb")
            nc.sync.dma_start(wm[:], t["w_qb"][m])
            ps = psA.tile([P, W], F32, tag="psA")
            for k in range(KB_QLR):
                mm(ps[:], wm[:, k, :], qaT[:, k, :],
                   start=(k == 0), stop=(k == KB_QLR - 1))
            if m < MB_NOPE:
                nc.scalar.activation(qnopeT[:, m, :], ps[:], COPY)
            else:
                j = m - MB_NOPE
                rotq = pc.tile([P, W], F32, tag="rotq")
                for h in (0, DR):
                    nc.vector.tensor_copy(rotq[h:h + 32, :],
                                          ps[h + 32:h + 64, :])
                    nc.vector.tensor_copy(rotq[h + 32:h + 64, :],
                                          ps[h:h + 32, :])
                nc.vector.tensor_tensor(rotq[:], rotq[:], sin2sp[:], MULT)
                tmp = pc.tile([P, W], F32, tag="tmpq")
                nc.vector.tensor_tensor(tmp[:], ps[:], cos2p[:], MULT)
                nc.vector.tensor_tensor(qpeT[:, j, :], tmp[:], rotq[:], ADD)

    # ------------- phase D: per 2-head group: V, knope, attention ----
    with tc.tile_pool(name="phD", bufs=2) as pd, \
         tc.tile_pool(name="phD_v", bufs=2) as pdv, \
         tc.tile_pool(name="phD_k", bufs=2) as pdk, \
         tc.tile_pool(name="phD_w", bufs=2) as pdw, \
         tc.tile_pool(name="probs", bufs=5) as pprob, \
         tc.tile_pool(name="psSc", bufs=4, space="PSUM") as psSc, \
         tc.tile_pool(name="psO", bufs=2, space="PSUM") as psO, \
         tc.tile_pool(name="psR", bufs=1, space="PSUM") as psR, \
         tc.tile_pool(name="psB2", bufs=1, space="PSUM") as psB2:
        from contextlib import ExitStack
        dctx = ExitStack()
        mask_pool = None
        if with_mask:
            mask_pool = dctx.enter_context(tc.tile_pool(name="maskp", bufs=4))

        # deferred normalization finish of the previous head, emitted
        # late so its PE bcast never stalls the in-order PE stream
        def finish_head(h, po, pr):
            prs = pd.tile([1, W], BF16, tag="prs")
            nc.scalar.activation(prs[:], pr[:], COPY)
            bc_ps = psB2.tile([P, W], F32, tag="bcd")
            mm(bc_ps[:], ones_row[:], prs[:], start=True, stop=True)
            rinv = pd.tile([P, W], F32, tag="rinv")
            nc.vector.reciprocal(rinv[:], bc_ps[:])
            nc.vector.tensor_tensor(oT[:, h, :], po[:], rinv[:], MULT)

        pending = None
        for g in range(NH // 2):
            # V for the 2 heads of this group: [128k, kb, 2*128]
            wv = pdw.tile([P, KB_CKV, 2 * DV], BF16, tag="wv")
            nc.sync.dma_start(wv[:], t["w_kvb_v"][g])
            v_sb = pdv.tile([P, KB_S, 2 * DV], BF16, tag="v")
            for kb in range(KB_S):
                psv = psSc.tile([P, W], F32, tag="pss")
                for kc in range(KB_CKV):
                    mm(psv[:, :2 * DV], ckT[:, kc, kb * P:(kb + 1) * P],
                       wv[:, kc, :], start=(kc == 0), stop=(kc == KB_CKV - 1))
                nc.scalar.activation(v_sb[:, kb, :], psv[:, :2 * DV], COPY)

            for hl in range(2):
                h = g * 2 + hl
                # knopeT for head h: [128 d, kb, 128 k]
                wkn = pdw.tile([P, KB_CKV, DN], BF16, tag="wkn")
                nc.sync.dma_start(wkn[:], t["w_kvb_kn"][h])
                knT = pdk.tile([P, KB_S, P], BF16, tag="knT")
                for nch in range(NCH):
                    psk = psSc.tile([P, W], F32, tag="pss")
                    for kc in range(KB_CKV):
                        mm(psk[:], wkn[:, kc, :],
                           ckT[:, kc, nch * W:(nch + 1) * W],
                           start=(kc == 0), stop=(kc == KB_CKV - 1))
                    nc.scalar.activation(
                        knT[:, nch * (W // P):(nch + 1) * (W // P), :],
                        psk[:], COPY)

                if pending is not None:
                    finish_head(*pending)
                    pending = None

                # attention for head h, software-pipelined over kb
                po = psO.tile([P, W], F32, tag="po")
                pr = psR.tile([1, W], F32, tag="pr")
                hp64 = hl * DR
                probs_q = []
                for kb in range(KB_S):
                    pss = psSc.tile([P, W], F32, tag="pss")
                    mm(pss[:], knT[:, kb, :], qnopeT[:, h, :],
                       start=True, stop=False)
                    mm(pss[:], kpe2[hp64:hp64 + DR, kb * P:(kb + 1) * P],
                       qpeT[hp64:hp64 + DR, g, :], start=False, stop=True)
                    probs = pprob.tile([P, W], BF16, tag="probs")
                    if with_mask:
                        mtile = mask_pool.tile([P, W], F32, tag="mt")
                        nc.sync.dma_start(
                            mtile[:], t["maskT"][kb * P:(kb + 1) * P, :])
                        pf = pprob.tile([P, W], F32, tag="probs_f")
                        nc.vector.scalar_tensor_tensor(
                            pf[:], pss[:], SCALE, mtile[:], MULT, ADD)
                        nc.scalar.activation(probs[:], pf[:], EXP)
                    else:
                        nc.scalar.activation(probs[:], pss[:], EXP,
                                             scale=SCALE)
                    probs_q.append((kb, probs))
                    if len(probs_q) == 4 or kb == KB_S - 1:
                        for kb2, pb2 in probs_q:
                            mm(po[:], v_sb[:, kb2, hl * DV:(hl + 1) * DV],
                               pb2[:], start=(kb2 == 0),
                               stop=(kb2 == KB_S - 1), skip_group_check=True)
                            mm(pr[:], ones_col[:], pb2[:],
                               start=(kb2 == 0), stop=(kb2 == KB_S - 1),
                               skip_group_check=True)
                        probs_q = []
                pending = (h, po, pr)
        finish_head(*pending)
        dctx.close()

    # ------------- phase E: o_proj -----------------------------------
    with tc.tile_pool(name="phE", bufs=2) as pe, \
         tc.tile_pool(name="phE_w", bufs=3) as pew, \
         tc.tile_pool(name="psA", bufs=3, space="PSUM") as psA:
        for m in range(MB_HID):
            wm = pew.tile([P, NH, P], BF16, tag="wo")
            nc.sync.dma_start(wm[:], t["w_o"][m])
            ps = psA.tile([P, W], F32, tag="psA")
            for k in range(NH):
                mm(ps[:], wm[:, k, :], oT[:, k, :],
                   start=(k == 0), stop=(k == NH - 1))
            osb = pe.tile([P, W], F32, tag="osb")
            nc.scalar.activation(osb[:], ps[:], COPY)
            nc.sync.dma_start(t["outT"][m * P:(m + 1) * P, :], osb[:])
    top.close()


def _build_program(with_mask):
    nc = bacc.Bacc("TRN2", target_bir_lowering=False, debug=False)
    t = {}

    def inp(name, shape, dt=BF16):
        t[name] = nc.dram_tensor(name, list(shape), dt,
                                 kind="ExternalInput").ap()

    inp("hs_pks", [P, KB_HID, S])
    inp("hs_pkp", [P, KB_HID, W])
    inp("w_qa", [MB_QLR, P, KB_HID, P])
    inp("w_qb", [MB_NOPE + MB_PE, P, KB_QLR, P])
    inp("w_kva", [P, MB_KVA, KB_HID, P])
    inp("w_kvb_kn", [NH, P, KB_CKV, DN])
    inp("w_kvb_v", [NH // 2, P, KB_CKV, 2 * DV])
    inp("w_o", [MB_HID, P, NH, P])
    inp("qa_ln_p", [P, MB_QLR], F32)
    inp("kva_ln_p", [P, KB_CKV], F32)
    inp("cos2p", [P, W], F32)
    inp("sin2sp", [P, W], F32)
    inp("cos2f", [P, S], F32)
    inp("sin2sf", [P, S], F32)
    if with_mask:
        inp("maskT", [S, W], F32)
    t["outT"] = nc.dram_tensor("outT", [HID, W], F32,
                               kind="ExternalOutput").ap()

    with tile.TileContext(nc) as tc:
        _emit(tc, t, with_mask)
    nc.compile()
    return nc


_PROG_CACHE = {}


def _get_program(with_mask):
    if with_mask not in _PROG_CACHE:
        _PROG_CACHE[with_mask] = _build_program(with_mask)
    return _PROG_CACHE[with_mask]


def _block4(w, mb, kb):
    """[kb*P, mb*P] -> [mb, P, kb, P] with W[m,p,k,c] = w[k*P+p, m*P+c]."""
    return np.ascontiguousarray(
        w.reshape(kb, P, mb, P).transpose(2, 1, 0, 3))


def make_in_maps(hidden_states, attention_mask, cos, sin, w_qa, qa_ln, w_qb,
                 w_kva, kva_ln, w_kvb, w_o, with_mask):
    f32 = np.float32
    c = np.ascontiguousarray

    w_qb_r = np.asarray(w_qb, f32).reshape(QLR, NH, DQK)
    w_qb_re = np.concatenate(
        [w_qb_r[:, :, :DN].reshape(QLR, NH * DN),
         w_qb_r[:, :, DN:].reshape(QLR, NH * DR)], axis=1)
    w_kva_pad = np.concatenate(
        [np.asarray(w_kva, f32), np.zeros((HID, P - DR), f32)], axis=1)
    kvb = np.asarray(w_kvb, f32).reshape(KB_CKV, P, NH, DN + DV)
    w_kvb_kn = c(kvb[:, :, :, :DN].transpose(2, 1, 0, 3)
                 .astype(NPBF))                        # [NH, P, KB_CKV, DN]
    w_kvb_v = c(kvb[:, :, :, DN:].reshape(KB_CKV, P, NH // 2, 2 * DV)
                .transpose(2, 1, 0, 3).astype(NPBF))   # [NH/2, P, kc, 256]

    qa_ln_p = c(np.asarray(qa_ln, f32).reshape(MB_QLR, P).T)
    kva_ln_p = c(np.asarray(kva_ln, f32).reshape(KB_CKV, P).T)

    cosT = np.asarray(cos, f32).T                      # [64, S]
    sinT = np.asarray(sin, f32).T
    sin_s = np.concatenate([-sinT[:DR // 2], sinT[DR // 2:]], axis=0)
    cos2 = c(np.concatenate([cosT, cosT], axis=0))     # [128, S]
    sin2s = c(np.concatenate([sin_s, sin_s], axis=0))

    shared = {
        "w_qa": _block4(np.asarray(w_qa, f32), MB_QLR, KB_HID).astype(NPBF),
        "w_qb": _block4(w_qb_re, MB_NOPE + MB_PE, KB_QLR).astype(NPBF),
        "w_kva": c(w_kva_pad.reshape(KB_HID, P, MB_KVA, P)
                   .transpose(1, 2, 0, 3).astype(NPBF)),
        "w_kvb_kn": w_kvb_kn,
        "w_kvb_v": w_kvb_v,
        "w_o": _block4(np.asarray(w_o, f32), MB_HID, KB_HID).astype(NPBF),
        "qa_ln_p": qa_ln_p,
        "kva_ln_p": kva_ln_p,
        "cos2f": cos2,
        "sin2sf": sin2s,
    }

    hs = np.asarray(hidden_states)
    am = np.asarray(attention_mask)
    in_maps = []
    for core in range(NCORES):
        b, pnl = divmod(core, NPANEL)
        q0 = pnl * W
        hsT = np.asarray(hs[b], f32).T                 # [HID, S]
        hs_pks = c(hsT.reshape(KB_HID, P, S).transpose(1, 0, 2)
                   .astype(NPBF))                      # [128, 16, S]
        m = dict(shared)
        m["hs_pks"] = hs_pks
        m["hs_pkp"] = c(hs_pks[:, :, q0:q0 + W])
        m["cos2p"] = c(cos2[:, q0:q0 + W])
        m["sin2sp"] = c(sin2s[:, q0:q0 + W])
        if with_mask:
            m["maskT"] = c(am[b, 0, q0:q0 + W, :].T.astype(f32))
        in_maps.append(m)
    return in_maps


def kernel(hidden_states, attention_mask, cos, sin, w_qa, qa_ln, w_qb,
           w_kva, kva_ln, w_kvb, w_o):
    global LAST_RESULT
    with_mask = bool(np.any(np.asarray(attention_mask) != 0))
    nc = _get_program(with_mask)
    in_maps = make_in_maps(hidden_states, attention_mask, cos, sin, w_qa,
                           qa_ln, w_qb, w_kva, kva_ln, w_kvb, w_o, with_mask)
    trace = os.environ.get("KERNEL_TRACE", "0") == "1"
    res = bass_utils.run_bass_kernel_spmd(
        nc, in_maps, core_ids=list(range(NCORES)), trace=trace)
    LAST_RESULT = res

    out = np.empty((B, S, HID), np.float32)
    for core in range(NCORES):
        b, pnl = divmod(core, NPANEL)
        q0 = pnl * W
        out[b, q0:q0 + W, :] = res.results[core]["outT"].T
    return out
